# revision 13
# baseline (speedup 1.0000x reference)
"""Trainium2 Bass kernel for the NeuralODE problem.

Math: 63 Euler steps of y' = -y + MLP(y), MLP = 2->128->128->128->2 with
leaky_relu(0.01). Output ys[n] = clip(y_n, -5, 5) for n = 0..63.

Key reformulation: let a1_n = W1 @ y_n + b1 (layer-1 preactivation,
[128, batch]).  Since y_{n+1} = s_n y_n + dt_n (W4 h3_n + b4) with
s_n = 1 - dt_n, the a1 recursion is

    a1_{n+1} = s_n a1_n + dt_n (W1 W4) h3_n + dt_n (b1 + W1 b4)

With e_n = prod_{j<n} s_j and P_n = a1-accumulator rescaled by 1/e_n,
P lives in PSUM and is updated purely by accumulating matmuls with the
dense rank-2 matrix Mt_n = (dt_n / e_{n+1}) (W1 W4)^T:

    P_0 = W1 @ y0,   P_{n+1} = P_n + Mt_n^T @ h3_n
    a1_n = e_n P_n + gamma_n          (gamma: host-computed [128] table)

h1_n = lrelu(a1_n) = ACT(Lrelu, in=P_n, scale=e_n, bias=gamma_n) -- the
rescale costs zero extra instructions, and the critical recursion cycle
is h1-ACT -> W2-mm -> h2-ACT -> W3-mm -> h3-ACT -> Mt-mm -> h1-ACT.

y_n itself is never materialized during the loop; it is linear in
(y0, o_0..o_62) with o_n = W4 h3_n, so o_n is computed as a side branch
(W4-mm -> DVE copy -> DMA into a [126, batch] history buffer) and the
whole clipped trajectory is reconstructed at the end with two matmuls
against host-precomputed coefficient matrices plus one fused clip.

Per core (1024 particles, 2 chunks of N=512), per step:
  PE : W2, W3, Mt (accumulate into P), W4   x2 chunks (all float32r)
  ACT: 3 Lrelu ops (PSUM->SBUF movers, bias/scale/rescale fused) x2 chunks
  DVE: 2 copies of o [2,512] PSUM->SBUF
  DMA: 2 small SBUF->SBUF copies of o into the history buffer
"""

import numpy as np

T = 64
B = 8192
D = 2
W = 128
CAP = 5.0
NCORES = 8
BC = B // NCORES  # 1024 particles per core
CH = 512  # matmul free-dim chunk (one PSUM bank)
NCH = BC // CH  # 2
NS = T - 1  # 63 integration steps


def _host_tables(ts, W1, b1, W4, b4):
    """Precompute (in float64) all per-step constants."""
    ts = ts.astype(np.float64)
    dts = ts[1:] - ts[:-1]  # [63]
    s = 1.0 - dts  # [63]
    e = np.ones(T, np.float64)  # e[n] = prod_{j<n} s[j]
    for n in range(NS):
        e[n + 1] = e[n] * s[n]

    W1d = W1.astype(np.float64)
    b1d = b1.astype(np.float64)
    c_base = b1d + W1d @ b4.astype(np.float64)  # [128]

    # gamma_n table: gamma_0 = b1; gamma_{n+1} = s_n gamma_n + dt_n c_base
    gam = np.zeros((W, NS), np.float64)
    g = b1d.copy()
    for n in range(NS):
        gam[:, n] = g
        g = s[n] * g + dts[n] * c_base

    # Mt table: lhsT of the P update, (dt_n / e_{n+1}) * (W1 @ W4).T
    M = W1d @ W4.astype(np.float64)  # [128, 128]
    mts = np.zeros((W, NS * W), np.float64)
    for n in range(NS):
        mts[:, n * W : (n + 1) * W] = (dts[n] / e[n + 1]) * M.T

    # Output reconstruction:
    #   y_n[d] = e_n y0[d] + sum_{j<n} dt_j (e_n/e_{j+1}) (o_j[d] + b4[d])
    # Output row m = 2n + d.  Amat row 2 carries the constant b4 part and
    # multiplies a ones-row appended to y0 on the host.  bmat2 holds, per
    # step j, the K=2 lhsT [2, 128] used to accumulate o_j into the
    # trajectory PSUM incrementally.
    b4d = b4.astype(np.float64)
    W4d = W4.astype(np.float64)
    Amat = np.zeros((D + 1, 2 * T), np.float64)
    bmat2 = np.zeros((D, NS * 2 * T), np.float64)
    for n in range(T):
        cb = sum(dts[j] * (e[n] / e[j + 1]) for j in range(n))
        for d in range(D):
            m = 2 * n + d
            Amat[d, m] = e[n]
            Amat[D, m] = cb * b4d[d]
            for j in range(n):
                bmat2[d, j * 2 * T + m] = dts[j] * (e[n] / e[j + 1])
    # Fold W4 in: cts[:, j*128:(j+1)*128] = W4^T @ bmat2_j, so the
    # trajectory accumulates directly from h3 (no o extraction at all).
    cts = np.zeros((W, NS * 2 * T), np.float64)
    for j in range(NS):
        cts[:, j * 2 * T : (j + 1) * 2 * T] = W4d.T @ bmat2[:, j * 2 * T : (j + 1) * 2 * T]

    return (
        gam.astype(np.float32),
        mts.astype(np.float32),
        Amat.astype(np.float32),
        cts.astype(np.float32),
        [float(x) for x in e],
    )



_LRELU_OP = None


def _register_lrelu_op():
    """Register a single-pass fused leaky-relu custom DVE op:
    out = max(z, z*imm2) with z = in0*s1 + s0 (s0 per-partition, s1/imm2
    literals). Uses the documented extension point (concourse.dve_ops.OPS);
    the uops sha is pinned to whatever this toolchain lowers to."""
    global _LRELU_OP
    if _LRELU_OP is not None:
        return _LRELU_OP
    import numpy as np

    import concourse.dve_ops as dve_ops
    from concourse.dve_spec import C0, C1, C2, Spec, Src0, lower, maxx
    from concourse.dve_uop import DveOpSpec

    for op in dve_ops.OPS:
        if op.name == "LRELU_AFFINE_ANT":
            _LRELU_OP = op
            return op

    z = Src0 * C1 + C0
    spec = Spec(
        body=maxx(z, z * C2),
        reference=lambda in0, in1, s0, s1, imm2: np.maximum(
            in0.astype(np.float32) * s1 + s0,
            (in0.astype(np.float32) * s1 + s0) * imm2,
        ).astype(np.float32),
    )
    row = dve_ops._CUSTOM_DVE_ROW_BASE + len(dve_ops.OPS)
    assert row < 0x20
    shas = {}
    for ver in ("v3", "v4"):
        shas[ver] = DveOpSpec(
            name="LRELU_AFFINE_ANT",
            opcode=row,
            uops=lower(spec, ver=ver),
            rd1_en=False,
        ).sha(ver)
    op = dve_ops.DveOp("LRELU_AFFINE_ANT", spec, subdim=False, uops_sha=shas)
    dve_ops.OPS.append(op)
    dve_ops._SUB_OPCODE_FOR_NAME[op.name] = row
    dve_ops.CUSTOM_DVE_SPECS[op.name] = spec
    _LRELU_OP = op
    return op


def _build_module(e_scales):
    import concourse.bacc as bacc
    import concourse.mybir as mybir
    import concourse.tile as tile

    F32 = mybir.dt.float32
    FMM = mybir.dt.float32r
    AF = mybir.ActivationFunctionType
    ALU = mybir.AluOpType

    nc = bacc.Bacc(None, target_bir_lowering=False, debug=False)

    # DRAM I/O. float32r maps to np.float32 on the host; tensors feeding
    # matmuls are declared float32r end-to-end so the BIR verifier sees
    # every producer rounding to fp32r.
    y0c_d = nc.dram_tensor("y0c", [D + 1, BC], FMM, kind="ExternalInput")
    w1t_d = nc.dram_tensor("w1t", [D, W], FMM, kind="ExternalInput")
    w2t_d = nc.dram_tensor("w2t", [W, W], FMM, kind="ExternalInput")
    w3t_d = nc.dram_tensor("w3t", [W, W], FMM, kind="ExternalInput")
    mts_d = nc.dram_tensor("mts", [W, NS * W], FMM, kind="ExternalInput")
    gam_d = nc.dram_tensor("gam", [W, NS], F32, kind="ExternalInput")
    b2_d = nc.dram_tensor("b2c", [W, 1], F32, kind="ExternalInput")
    b3_d = nc.dram_tensor("b3c", [W, 1], F32, kind="ExternalInput")
    amat_d = nc.dram_tensor("amat", [D + 1, 2 * T], FMM, kind="ExternalInput")
    bmat_d = nc.dram_tensor("bmat", [W, NS * 2 * T], FMM, kind="ExternalInput")
    yout_d = nc.dram_tensor("yout", [2 * T, BC], F32, kind="ExternalOutput")

    with tile.TileContext(nc) as tc:
        with (
            tc.tile_pool(name="const", bufs=1) as cpool,
            tc.tile_pool(name="h1", bufs=3) as h1pool,
            tc.tile_pool(name="h2", bufs=3) as h2pool,
            tc.tile_pool(name="h3", bufs=3) as h3pool,
            tc.tile_pool(name="yo", bufs=2) as ypool,
            tc.tile_pool(name="pp", bufs=1, space="PSUM") as ppsum,
            tc.tile_pool(name="py", bufs=1, space="PSUM") as ypsum_pool,
            tc.tile_pool(name="pa0l", bufs=1, space="PSUM") as papool0l,
            tc.tile_pool(name="pa0h", bufs=1, space="PSUM") as papool0h,
            tc.tile_pool(name="pa1l", bufs=1, space="PSUM") as papool1l,
            tc.tile_pool(name="pa1h", bufs=1, space="PSUM") as papool1h,
        ):
            papool = [[papool0l, papool0h], [papool1l, papool1h]]
            # ---- constants into SBUF ----
            y0t = cpool.tile([D + 1, BC], FMM)
            w1t = cpool.tile([D, W], FMM)
            w2t = cpool.tile([W, W], FMM)
            w3t = cpool.tile([W, W], FMM)
            mts = cpool.tile([W, NS * W], FMM)
            gam = cpool.tile([W, NS], F32)
            b2c = cpool.tile([W, 1], F32)
            b3c = cpool.tile([W, 1], F32)
            amat = cpool.tile([D + 1, 2 * T], FMM)
            bmat = cpool.tile([W, NS * 2 * T], FMM)
            for i, (t_sb, t_dr) in enumerate((
                (y0t, y0c_d), (w1t, w1t_d), (w2t, w2t_d), (w3t, w3t_d),
                (gam, gam_d), (b2c, b2_d),
                (b3c, b3_d), (amat, amat_d),
            )):
                (nc.sync if i % 2 == 0 else nc.gpsimd).dma_start(t_sb[:], t_dr[:])
            # the two big tables: quarter them and spread across both queues
            for big_sb, big_dr, ncols in ((mts, mts_d, NS * W), (bmat, bmat_d, NS * 2 * T)):
                q = ncols // 4
                for i in range(4):
                    sl = slice(i * q, (i + 1) * q) if i < 3 else slice(3 * q, ncols)
                    (nc.sync if i % 2 == 0 else nc.gpsimd).dma_start(
                        big_sb[:, sl], big_dr[:, sl]
                    )

            # ---- persistent PSUM state: P (the rescaled a1 accumulator) ----
            P = [
                ppsum.tile([W, CH], F32, name=f"P{c}", tag=f"P{c}")
                for c in range(NCH)
            ]
            Y = [
                ypsum_pool.tile([2 * T, CH], F32, name=f"Y{c}", tag=f"Y{c}")
                for c in range(NCH)
            ]
            for c in range(NCH):
                nc.tensor.matmul(
                    P[c][:], w1t[:], y0t[0:D, c * CH : (c + 1) * CH],
                    start=True, stop=True,
                )
                nc.tensor.matmul(
                    Y[c][:], amat[:], y0t[:, c * CH : (c + 1) * CH],
                    start=True, stop=True,
                )

            # ---- integration loop ----
            lrelu_op = _register_lrelu_op()

            def dve_lrelu(out_ap, in_ap, bias_ap, scale):
                nc.vector._custom_dve(
                    lrelu_op, out=out_ap, in0=in_ap,
                    s0=bias_ap, s1=float(scale), imm2=0.01,
                )

            HF = CH // 2  # 256: lo/hi column halves of each chunk

            for n in range(NS):
                en = e_scales[n]
                for c in range(NCH):
                    h1 = h1pool.tile([W, CH], FMM, tag="h1")
                    # h1 also split lo(ACT)/hi(DVE): the halves read disjoint
                    # column ranges of the P bank and phase-shift naturally.
                    nc.scalar.activation(
                        h1[:, 0 : CH // 2], P[c][:, 0 : CH // 2], AF.Lrelu,
                        bias=gam[:, n : n + 1], scale=float(en), alpha=0.01,
                    )
                    dve_lrelu(
                        h1[:, CH // 2 : CH], P[c][:, CH // 2 : CH],
                        gam[:, n : n + 1], en,
                    )
                    # layers 2 and 3 run as two independent column streams
                    # (lo on ACT, hi on DVE) in separate PSUM banks, halving
                    # the lrelu latency on the recursion cycle.
                    h2 = h2pool.tile([W, CH], FMM, tag="h2")
                    h3 = h3pool.tile([W, CH], FMM, tag="h3")
                    for half in range(2):
                        hs = slice(half * HF, (half + 1) * HF)
                        a2 = papool[c][half].tile(
                            [W, HF], F32, name=f"a2_{n}_{c}_{half}", tag="a"
                        )
                        nc.tensor.matmul(
                            a2[:], w2t[:], h1[:, hs], start=True, stop=True
                        )
                        if half == 0:
                            nc.scalar.activation(
                                h2[:, hs], a2[:], AF.Lrelu,
                                bias=b2c[:], scale=1.0, alpha=0.01,
                            )
                        else:
                            dve_lrelu(h2[:, hs], a2[:], b2c[:], 1.0)
                        a3 = papool[c][half].tile(
                            [W, HF], F32, name=f"a3_{n}_{c}_{half}", tag="a"
                        )
                        nc.tensor.matmul(
                            a3[:], w3t[:], h2[:, hs], start=True, stop=True
                        )
                        if half == 0:
                            nc.scalar.activation(
                                h3[:, hs], a3[:], AF.Lrelu,
                                bias=b3c[:], scale=1.0, alpha=0.01,
                            )
                        else:
                            dve_lrelu(h3[:, hs], a3[:], b3c[:], 1.0)
                    # P update first: it is on the critical recursion cycle.
                    if n < NS - 1:
                        nc.tensor.matmul(
                            P[c][:], mts[:, n * W : (n + 1) * W], h3[:],
                            start=False, stop=True,
                        )
                    # trajectory accumulation straight from h3 (W4 folded in)
                    nc.tensor.matmul(
                        Y[c][:], bmat[:, n * 2 * T : (n + 1) * 2 * T], h3[:],
                        start=False, stop=True,
                    )

            # ---- clip and store the trajectory ----
            for c in range(NCH):
                cs = slice(c * CH, (c + 1) * CH)
                yo = ypool.tile([2 * T, CH], F32, tag="yo")
                nc.vector.tensor_scalar(
                    yo[:], Y[c][:], -CAP, CAP, ALU.max, ALU.min
                )
                nc.sync.dma_start(yout_d[:, cs], yo[:])

    nc.compile()
    return nc


_NC_CACHE = None


def _get_module(e_scales):
    global _NC_CACHE
    if _NC_CACHE is None:
        _NC_CACHE = _build_module(e_scales)
    return _NC_CACHE


def kernel(ts, y0, W1, b1, W2, b2, W3, b3, W4, b4):
    ts = np.asarray(ts, np.float32)
    y0 = np.asarray(y0, np.float32)
    W1 = np.asarray(W1, np.float32)
    b1 = np.asarray(b1, np.float32)
    W2 = np.asarray(W2, np.float32)
    b2 = np.asarray(b2, np.float32)
    W3 = np.asarray(W3, np.float32)
    b3 = np.asarray(b3, np.float32)
    W4 = np.asarray(W4, np.float32)
    b4 = np.asarray(b4, np.float32)

    gam, mts, Amat, Bmat, e_scales = _host_tables(ts, W1, b1, W4, b4)
    nc = _get_module(e_scales)

    y0t_all = np.ascontiguousarray(y0.T)  # [2, 8192]
    shared = {
        "w1t": np.ascontiguousarray(W1.T),
        "w2t": np.ascontiguousarray(W2.T),
        "w3t": np.ascontiguousarray(W3.T),
        "mts": mts,
        "gam": gam,
        "b2c": b2.reshape(W, 1).copy(),
        "b3c": b3.reshape(W, 1).copy(),
        "amat": Amat,
        "bmat": Bmat,
    }
    in_maps = []
    for i in range(NCORES):
        m = dict(shared)
        m["y0c"] = np.ascontiguousarray(
            np.vstack(
                [y0t_all[:, i * BC : (i + 1) * BC], np.ones((1, BC), np.float32)]
            )
        )
        in_maps.append(m)

    from concourse.bass_utils import run_bass_kernel_spmd

    res = run_bass_kernel_spmd(nc, in_maps, core_ids=list(range(NCORES)))

    ys = np.empty((T, B, D), np.float32)
    for i in range(NCORES):
        arr = res.results[i]["yout"]  # [128, 1024]
        ys[:, i * BC : (i + 1) * BC, :] = arr.reshape(T, D, BC).transpose(0, 2, 1)
    return ys


# revision 14
# speedup vs baseline: 1.0454x; 1.0454x over previous
"""Trainium2 Bass kernel for the NeuralODE problem.

Math: 63 Euler steps of y' = -y + MLP(y), MLP = 2->128->128->128->2 with
leaky_relu(0.01). Output ys[n] = clip(y_n, -5, 5) for n = 0..63.

Key reformulation: let a1_n = W1 @ y_n + b1 (layer-1 preactivation,
[128, batch]).  Since y_{n+1} = s_n y_n + dt_n (W4 h3_n + b4) with
s_n = 1 - dt_n, the a1 recursion is

    a1_{n+1} = s_n a1_n + dt_n (W1 W4) h3_n + dt_n (b1 + W1 b4)

With e_n = prod_{j<n} s_j and P_n = a1-accumulator rescaled by 1/e_n,
P lives in PSUM and is updated purely by accumulating matmuls with the
dense rank-2 matrix Mt_n = (dt_n / e_{n+1}) (W1 W4)^T:

    P_0 = W1 @ y0,   P_{n+1} = P_n + Mt_n^T @ h3_n
    a1_n = e_n P_n + gamma_n          (gamma: host-computed [128] table)

h1_n = lrelu(a1_n) = ACT(Lrelu, in=P_n, scale=e_n, bias=gamma_n) -- the
rescale costs zero extra instructions, and the critical recursion cycle
is h1-ACT -> W2-mm -> h2-ACT -> W3-mm -> h3-ACT -> Mt-mm -> h1-ACT.

y_n itself is never materialized during the loop; it is linear in
(y0, o_0..o_62) with o_n = W4 h3_n, so o_n is computed as a side branch
(W4-mm -> DVE copy -> DMA into a [126, batch] history buffer) and the
whole clipped trajectory is reconstructed at the end with two matmuls
against host-precomputed coefficient matrices plus one fused clip.

Per core (1024 particles, 2 chunks of N=512), per step:
  PE : W2, W3, Mt (accumulate into P), W4   x2 chunks (all float32r)
  ACT: 3 Lrelu ops (PSUM->SBUF movers, bias/scale/rescale fused) x2 chunks
  DVE: 2 copies of o [2,512] PSUM->SBUF
  DMA: 2 small SBUF->SBUF copies of o into the history buffer
"""

import numpy as np

T = 64
B = 8192
D = 2
W = 128
CAP = 5.0
NCORES = 8
BC = B // NCORES  # 1024 particles per core
CH = 512  # matmul free-dim chunk (one PSUM bank)
NCH = BC // CH  # 2
NS = T - 1  # 63 integration steps


def _host_tables(ts, W1, b1, W4, b4):
    """Precompute (in float64) all per-step constants."""
    ts = ts.astype(np.float64)
    dts = ts[1:] - ts[:-1]  # [63]
    s = 1.0 - dts  # [63]
    e = np.ones(T, np.float64)  # e[n] = prod_{j<n} s[j]
    for n in range(NS):
        e[n + 1] = e[n] * s[n]

    W1d = W1.astype(np.float64)
    b1d = b1.astype(np.float64)
    c_base = b1d + W1d @ b4.astype(np.float64)  # [128]

    # gamma_n table: gamma_0 = b1; gamma_{n+1} = s_n gamma_n + dt_n c_base
    gam = np.zeros((W, NS), np.float64)
    g = b1d.copy()
    for n in range(NS):
        gam[:, n] = g
        g = s[n] * g + dts[n] * c_base

    # Mt table: lhsT of the P update, (dt_n / e_{n+1}) * (W1 @ W4).T
    M = W1d @ W4.astype(np.float64)  # [128, 128]
    mts = np.zeros((W, NS * W), np.float64)
    for n in range(NS):
        mts[:, n * W : (n + 1) * W] = (dts[n] / e[n + 1]) * M.T

    # Output reconstruction:
    #   y_n[d] = e_n y0[d] + sum_{j<n} dt_j (e_n/e_{j+1}) (o_j[d] + b4[d])
    # Output row m = 2n + d.  Amat row 2 carries the constant b4 part and
    # multiplies a ones-row appended to y0 on the host.  bmat2 holds, per
    # step j, the K=2 lhsT [2, 128] used to accumulate o_j into the
    # trajectory PSUM incrementally.
    b4d = b4.astype(np.float64)
    W4d = W4.astype(np.float64)
    Amat = np.zeros((D + 1, 2 * T), np.float64)
    bmat2 = np.zeros((D, NS * 2 * T), np.float64)
    for n in range(T):
        cb = sum(dts[j] * (e[n] / e[j + 1]) for j in range(n))
        for d in range(D):
            m = 2 * n + d
            Amat[d, m] = e[n]
            Amat[D, m] = cb * b4d[d]
            for j in range(n):
                bmat2[d, j * 2 * T + m] = dts[j] * (e[n] / e[j + 1])
    # Fold W4 in: cts[:, j*128:(j+1)*128] = W4^T @ bmat2_j, so the
    # trajectory accumulates directly from h3 (no o extraction at all).
    cts = np.zeros((W, NS * 2 * T), np.float64)
    for j in range(NS):
        cts[:, j * 2 * T : (j + 1) * 2 * T] = W4d.T @ bmat2[:, j * 2 * T : (j + 1) * 2 * T]

    return (
        gam.astype(np.float32),
        mts.astype(np.float32),
        Amat.astype(np.float32),
        cts.astype(np.float32),
        [float(x) for x in e],
    )



_LRELU_OP = None


def _register_lrelu_op():
    """Register a single-pass fused leaky-relu custom DVE op:
    out = max(z, z*imm2) with z = in0*s1 + s0 (s0 per-partition, s1/imm2
    literals). Uses the documented extension point (concourse.dve_ops.OPS);
    the uops sha is pinned to whatever this toolchain lowers to."""
    global _LRELU_OP
    if _LRELU_OP is not None:
        return _LRELU_OP
    import numpy as np

    import concourse.dve_ops as dve_ops
    from concourse.dve_spec import C0, C1, C2, Spec, Src0, lower, maxx
    from concourse.dve_uop import DveOpSpec

    for op in dve_ops.OPS:
        if op.name == "LRELU_AFFINE_ANT":
            _LRELU_OP = op
            return op

    z = Src0 * C1 + C0
    spec = Spec(
        body=maxx(z, z * C2),
        reference=lambda in0, in1, s0, s1, imm2: np.maximum(
            in0.astype(np.float32) * s1 + s0,
            (in0.astype(np.float32) * s1 + s0) * imm2,
        ).astype(np.float32),
    )
    row = dve_ops._CUSTOM_DVE_ROW_BASE + len(dve_ops.OPS)
    assert row < 0x20
    shas = {}
    for ver in ("v3", "v4"):
        shas[ver] = DveOpSpec(
            name="LRELU_AFFINE_ANT",
            opcode=row,
            uops=lower(spec, ver=ver),
            rd1_en=False,
        ).sha(ver)
    op = dve_ops.DveOp("LRELU_AFFINE_ANT", spec, subdim=False, uops_sha=shas)
    dve_ops.OPS.append(op)
    dve_ops._SUB_OPCODE_FOR_NAME[op.name] = row
    dve_ops.CUSTOM_DVE_SPECS[op.name] = spec
    _LRELU_OP = op
    return op


def _build_module(e_scales):
    import concourse.bacc as bacc
    import concourse.mybir as mybir
    import concourse.tile as tile

    F32 = mybir.dt.float32
    FMM = mybir.dt.float32r
    AF = mybir.ActivationFunctionType
    ALU = mybir.AluOpType

    nc = bacc.Bacc(None, target_bir_lowering=False, debug=False)

    # DRAM I/O. float32r maps to np.float32 on the host; tensors feeding
    # matmuls are declared float32r end-to-end so the BIR verifier sees
    # every producer rounding to fp32r.
    y0c_d = nc.dram_tensor("y0c", [D + 1, BC], FMM, kind="ExternalInput")
    w1t_d = nc.dram_tensor("w1t", [D, W], FMM, kind="ExternalInput")
    w2t_d = nc.dram_tensor("w2t", [W, W], FMM, kind="ExternalInput")
    w3t_d = nc.dram_tensor("w3t", [W, W], FMM, kind="ExternalInput")
    mts_d = nc.dram_tensor("mts", [W, NS * W], FMM, kind="ExternalInput")
    gam_d = nc.dram_tensor("gam", [W, NS], F32, kind="ExternalInput")
    b2_d = nc.dram_tensor("b2c", [W, 1], F32, kind="ExternalInput")
    b3_d = nc.dram_tensor("b3c", [W, 1], F32, kind="ExternalInput")
    amat_d = nc.dram_tensor("amat", [D + 1, 2 * T], FMM, kind="ExternalInput")
    bmat_d = nc.dram_tensor("bmat", [W, NS * 2 * T], FMM, kind="ExternalInput")
    yout_d = nc.dram_tensor("yout", [2 * T, BC], F32, kind="ExternalOutput")

    with tile.TileContext(nc) as tc:
        with (
            tc.tile_pool(name="const", bufs=1) as cpool,
            tc.tile_pool(name="h1", bufs=3) as h1pool,
            tc.tile_pool(name="h2", bufs=3) as h2pool,
            tc.tile_pool(name="h3", bufs=3) as h3pool,
            tc.tile_pool(name="yo", bufs=2) as ypool,
            tc.tile_pool(name="pp", bufs=1, space="PSUM") as ppsum,
            tc.tile_pool(name="py", bufs=1, space="PSUM") as ypsum_pool,
            tc.tile_pool(name="pa0l", bufs=1, space="PSUM") as papool0l,
            tc.tile_pool(name="pa0h", bufs=1, space="PSUM") as papool0h,
            tc.tile_pool(name="pa1l", bufs=1, space="PSUM") as papool1l,
            tc.tile_pool(name="pa1h", bufs=1, space="PSUM") as papool1h,
        ):
            papool = [[papool0l, papool0h], [papool1l, papool1h]]
            # ---- constants into SBUF ----
            y0t = cpool.tile([D + 1, BC], FMM)
            w1t = cpool.tile([D, W], FMM)
            w2t = cpool.tile([W, W], FMM)
            w3t = cpool.tile([W, W], FMM)
            mts = cpool.tile([W, NS * W], FMM)
            gam = cpool.tile([W, NS], F32)
            b2c = cpool.tile([W, 1], F32)
            b3c = cpool.tile([W, 1], F32)
            amat = cpool.tile([D + 1, 2 * T], FMM)
            bmat = cpool.tile([W, NS * 2 * T], FMM)
            for i, (t_sb, t_dr) in enumerate((
                (y0t, y0c_d), (w1t, w1t_d), (w2t, w2t_d), (w3t, w3t_d),
                (gam, gam_d), (b2c, b2_d),
                (b3c, b3_d), (amat, amat_d),
            )):
                (nc.sync if i % 2 == 0 else nc.gpsimd).dma_start(t_sb[:], t_dr[:])
            # the two big tables: quarter them and spread across both queues
            for big_sb, big_dr, ncols in ((mts, mts_d, NS * W), (bmat, bmat_d, NS * 2 * T)):
                q = ncols // 4
                for i in range(4):
                    sl = slice(i * q, (i + 1) * q) if i < 3 else slice(3 * q, ncols)
                    (nc.sync if i % 2 == 0 else nc.gpsimd).dma_start(
                        big_sb[:, sl], big_dr[:, sl]
                    )

            # ---- persistent PSUM state: P (the rescaled a1 accumulator) ----
            P = [
                ppsum.tile([W, CH], F32, name=f"P{c}", tag=f"P{c}")
                for c in range(NCH)
            ]
            Y = [
                ypsum_pool.tile([2 * T, CH], F32, name=f"Y{c}", tag=f"Y{c}")
                for c in range(NCH)
            ]
            for c in range(NCH):
                nc.tensor.matmul(
                    P[c][:], w1t[:], y0t[0:D, c * CH : (c + 1) * CH],
                    start=True, stop=True,
                )
                nc.tensor.matmul(
                    Y[c][:], amat[:], y0t[:, c * CH : (c + 1) * CH],
                    start=True, stop=True,
                )

            # ---- integration loop ----
            lrelu_op = _register_lrelu_op()

            def dve_lrelu(out_ap, in_ap, bias_ap, scale):
                nc.vector._custom_dve(
                    lrelu_op, out=out_ap, in0=in_ap,
                    s0=bias_ap, s1=float(scale), imm2=0.01,
                )

            HF = CH // 2  # 256: lo/hi column halves of each chunk

            for n in range(NS):
                en = e_scales[n]
                ymms = []
                for c in range(NCH):
                    h1 = h1pool.tile([W, CH], FMM, tag="h1")
                    if c == 0:
                        nc.scalar.activation(
                            h1[:], P[c][:], AF.Lrelu,
                            bias=gam[:, n : n + 1], scale=float(en), alpha=0.01,
                        )
                    else:
                        dve_lrelu(h1[:], P[c][:], gam[:, n : n + 1], en)
                    # layers 2 and 3 run as two independent column streams
                    # (lo on ACT, hi on DVE) in separate PSUM banks, halving
                    # the lrelu latency on the recursion cycle.
                    h2 = h2pool.tile([W, CH], FMM, tag="h2")
                    h3 = h3pool.tile([W, CH], FMM, tag="h3")
                    for half in range(2):
                        hs = slice(half * HF, (half + 1) * HF)
                        a2 = papool[c][half].tile(
                            [W, HF], F32, name=f"a2_{n}_{c}_{half}", tag="a"
                        )
                        nc.tensor.matmul(
                            a2[:], w2t[:], h1[:, hs], start=True, stop=True
                        )
                        if half == 0:
                            nc.scalar.activation(
                                h2[:, hs], a2[:], AF.Lrelu,
                                bias=b2c[:], scale=1.0, alpha=0.01,
                            )
                        else:
                            dve_lrelu(h2[:, hs], a2[:], b2c[:], 1.0)
                        a3 = papool[c][half].tile(
                            [W, HF], F32, name=f"a3_{n}_{c}_{half}", tag="a"
                        )
                        nc.tensor.matmul(
                            a3[:], w3t[:], h2[:, hs], start=True, stop=True
                        )
                        if half == 0:
                            nc.scalar.activation(
                                h3[:, hs], a3[:], AF.Lrelu,
                                bias=b3c[:], scale=1.0, alpha=0.01,
                            )
                        else:
                            dve_lrelu(h3[:, hs], a3[:], b3c[:], 1.0)
                    # P update first: it is on the critical recursion
                    # cycle.  Split into halves so each half only waits on
                    # its own h3 column stream.
                    if n < NS - 1:
                        for half in range(2):
                            hs = slice(half * HF, (half + 1) * HF)
                            nc.tensor.matmul(
                                P[c][:, hs], mts[:, n * W : (n + 1) * W],
                                h3[:, hs], start=False, stop=True,
                            )
                    ymms.append((Y[c], h3))
                # trajectory accumulation straight from h3 (W4 folded in);
                # emitted after the cycle-critical matmuls of both chunks.
                for yc, h3c in ymms:
                    nc.tensor.matmul(
                        yc[:], bmat[:, n * 2 * T : (n + 1) * 2 * T], h3c[:],
                        start=False, stop=True,
                    )

            # ---- clip and store the trajectory ----
            for c in range(NCH):
                cs = slice(c * CH, (c + 1) * CH)
                yo = ypool.tile([2 * T, CH], F32, tag="yo")
                nc.vector.tensor_scalar(
                    yo[:], Y[c][:], -CAP, CAP, ALU.max, ALU.min
                )
                nc.sync.dma_start(yout_d[:, cs], yo[:])

    nc.compile()
    return nc


_NC_CACHE = None


def _get_module(e_scales):
    global _NC_CACHE
    if _NC_CACHE is None:
        _NC_CACHE = _build_module(e_scales)
    return _NC_CACHE


def kernel(ts, y0, W1, b1, W2, b2, W3, b3, W4, b4):
    ts = np.asarray(ts, np.float32)
    y0 = np.asarray(y0, np.float32)
    W1 = np.asarray(W1, np.float32)
    b1 = np.asarray(b1, np.float32)
    W2 = np.asarray(W2, np.float32)
    b2 = np.asarray(b2, np.float32)
    W3 = np.asarray(W3, np.float32)
    b3 = np.asarray(b3, np.float32)
    W4 = np.asarray(W4, np.float32)
    b4 = np.asarray(b4, np.float32)

    gam, mts, Amat, Bmat, e_scales = _host_tables(ts, W1, b1, W4, b4)
    nc = _get_module(e_scales)

    y0t_all = np.ascontiguousarray(y0.T)  # [2, 8192]
    shared = {
        "w1t": np.ascontiguousarray(W1.T),
        "w2t": np.ascontiguousarray(W2.T),
        "w3t": np.ascontiguousarray(W3.T),
        "mts": mts,
        "gam": gam,
        "b2c": b2.reshape(W, 1).copy(),
        "b3c": b3.reshape(W, 1).copy(),
        "amat": Amat,
        "bmat": Bmat,
    }
    in_maps = []
    for i in range(NCORES):
        m = dict(shared)
        m["y0c"] = np.ascontiguousarray(
            np.vstack(
                [y0t_all[:, i * BC : (i + 1) * BC], np.ones((1, BC), np.float32)]
            )
        )
        in_maps.append(m)

    from concourse.bass_utils import run_bass_kernel_spmd

    res = run_bass_kernel_spmd(nc, in_maps, core_ids=list(range(NCORES)))

    ys = np.empty((T, B, D), np.float32)
    for i in range(NCORES):
        arr = res.results[i]["yout"]  # [128, 1024]
        ys[:, i * BC : (i + 1) * BC, :] = arr.reshape(T, D, BC).transpose(0, 2, 1)
    return ys


# revision 24
# speedup vs baseline: 1.0676x; 1.0213x over previous
"""Trainium2 Bass kernel for the NeuralODE problem.

Math: 63 Euler steps of y' = -y + MLP(y), MLP = 2->128->128->128->2 with
leaky_relu(0.01). Output ys[n] = clip(y_n, -5, 5) for n = 0..63.

Key reformulation: let a1_n = W1 @ y_n + b1 (layer-1 preactivation,
[128, batch]).  Since y_{n+1} = s_n y_n + dt_n (W4 h3_n + b4) with
s_n = 1 - dt_n, the a1 recursion is

    a1_{n+1} = s_n a1_n + dt_n (W1 W4) h3_n + dt_n (b1 + W1 b4)

With e_n = prod_{j<n} s_j and P_n = a1-accumulator rescaled by 1/e_n,
P lives in PSUM and is updated purely by accumulating matmuls with the
dense rank-2 matrix Mt_n = (dt_n / e_{n+1}) (W1 W4)^T:

    P_0 = W1 @ y0,   P_{n+1} = P_n + Mt_n^T @ h3_n
    a1_n = e_n P_n + gamma_n          (gamma: host-computed [128] table)

h1_n = lrelu(a1_n) = ACT(Lrelu, in=P_n, scale=e_n, bias=gamma_n) -- the
rescale costs zero extra instructions, and the critical recursion cycle
is h1-ACT -> W2-mm -> h2-ACT -> W3-mm -> h3-ACT -> Mt-mm -> h1-ACT.

y_n itself is never materialized during the loop; it is linear in
(y0, o_0..o_62) with o_n = W4 h3_n, so o_n is computed as a side branch
(W4-mm -> DVE copy -> DMA into a [126, batch] history buffer) and the
whole clipped trajectory is reconstructed at the end with two matmuls
against host-precomputed coefficient matrices plus one fused clip.

Per core (1024 particles, 2 chunks of N=512), per step:
  PE : W2, W3, Mt (accumulate into P), W4   x2 chunks (all float32r)
  ACT: 3 Lrelu ops (PSUM->SBUF movers, bias/scale/rescale fused) x2 chunks
  DVE: 2 copies of o [2,512] PSUM->SBUF
  DMA: 2 small SBUF->SBUF copies of o into the history buffer
"""

import numpy as np

T = 64
B = 8192
D = 2
W = 128
CAP = 5.0
NCORES = 8
BC = B // NCORES  # 1024 particles per core
CH = 512  # matmul free-dim chunk (one PSUM bank)
NCH = BC // CH  # 2
NS = T - 1  # 63 integration steps


def _host_tables(ts, W1, b1, W4, b4):
    """Precompute (in float64) all per-step constants."""
    ts = ts.astype(np.float64)
    dts = ts[1:] - ts[:-1]  # [63]
    s = 1.0 - dts  # [63]
    e = np.ones(T, np.float64)  # e[n] = prod_{j<n} s[j]
    for n in range(NS):
        e[n + 1] = e[n] * s[n]

    W1d = W1.astype(np.float64)
    b1d = b1.astype(np.float64)
    c_base = b1d + W1d @ b4.astype(np.float64)  # [128]

    # gamma_n table: gamma_0 = b1; gamma_{n+1} = s_n gamma_n + dt_n c_base
    gam = np.zeros((W, NS), np.float64)
    g = b1d.copy()
    for n in range(NS):
        gam[:, n] = g
        g = s[n] * g + dts[n] * c_base

    # Mt table: lhsT of the P update, (dt_n / e_{n+1}) * (W1 @ W4).T
    M = W1d @ W4.astype(np.float64)  # [128, 128]
    mts = np.zeros((W, NS * W), np.float64)
    for n in range(NS):
        mts[:, n * W : (n + 1) * W] = (dts[n] / e[n + 1]) * M.T

    # Output reconstruction:
    #   y_n[d] = e_n y0[d] + sum_{j<n} dt_j (e_n/e_{j+1}) (o_j[d] + b4[d])
    # Output row m = 2n + d.  Amat row 2 carries the constant b4 part and
    # multiplies a ones-row appended to y0 on the host.  bmat2 holds, per
    # step j, the K=2 lhsT [2, 128] used to accumulate o_j into the
    # trajectory PSUM incrementally.
    b4d = b4.astype(np.float64)
    W4d = W4.astype(np.float64)
    Amat = np.zeros((D + 1, 2 * T), np.float64)
    bmat2 = np.zeros((D, NS * 2 * T), np.float64)
    for n in range(T):
        cb = sum(dts[j] * (e[n] / e[j + 1]) for j in range(n))
        for d in range(D):
            m = 2 * n + d
            Amat[d, m] = e[n]
            Amat[D, m] = cb * b4d[d]
            for j in range(n):
                bmat2[d, j * 2 * T + m] = dts[j] * (e[n] / e[j + 1])
    # Fold W4 in: cts[:, j*128:(j+1)*128] = W4^T @ bmat2_j, so the
    # trajectory accumulates directly from h3 (no o extraction at all).
    cts = np.zeros((W, NS * 2 * T), np.float64)
    for j in range(NS):
        cts[:, j * 2 * T : (j + 1) * 2 * T] = W4d.T @ bmat2[:, j * 2 * T : (j + 1) * 2 * T]

    return (
        gam.astype(np.float32),
        mts.astype(np.float32),
        Amat.astype(np.float32),
        cts.astype(np.float32),
        [float(x) for x in e],
    )



_LRELU_OP = None


def _register_lrelu_op():
    """Register a single-pass fused leaky-relu custom DVE op:
    out = max(z, z*imm2) with z = in0*s1 + s0 (s0 per-partition, s1/imm2
    literals). Uses the documented extension point (concourse.dve_ops.OPS);
    the uops sha is pinned to whatever this toolchain lowers to."""
    global _LRELU_OP
    if _LRELU_OP is not None:
        return _LRELU_OP
    import numpy as np

    import concourse.dve_ops as dve_ops
    from concourse.dve_spec import C0, C1, C2, Spec, Src0, lower, maxx
    from concourse.dve_uop import DveOpSpec

    for op in dve_ops.OPS:
        if op.name == "LRELU_AFFINE_ANT":
            _LRELU_OP = op
            return op

    z = Src0 * C1 + C0
    spec = Spec(
        body=maxx(z, z * C2),
        reference=lambda in0, in1, s0, s1, imm2: np.maximum(
            in0.astype(np.float32) * s1 + s0,
            (in0.astype(np.float32) * s1 + s0) * imm2,
        ).astype(np.float32),
    )
    row = dve_ops._CUSTOM_DVE_ROW_BASE + len(dve_ops.OPS)
    assert row < 0x20
    shas = {}
    for ver in ("v3", "v4"):
        shas[ver] = DveOpSpec(
            name="LRELU_AFFINE_ANT",
            opcode=row,
            uops=lower(spec, ver=ver),
            rd1_en=False,
        ).sha(ver)
    op = dve_ops.DveOp("LRELU_AFFINE_ANT", spec, subdim=False, uops_sha=shas)
    dve_ops.OPS.append(op)
    dve_ops._SUB_OPCODE_FOR_NAME[op.name] = row
    dve_ops.CUSTOM_DVE_SPECS[op.name] = spec
    _LRELU_OP = op
    return op


def _build_module(e_scales):
    import concourse.bacc as bacc
    import concourse.mybir as mybir
    import concourse.tile as tile

    F32 = mybir.dt.float32
    FMM = mybir.dt.float32r
    AF = mybir.ActivationFunctionType
    ALU = mybir.AluOpType

    nc = bacc.Bacc(None, target_bir_lowering=False, debug=False)

    # DRAM I/O. float32r maps to np.float32 on the host; tensors feeding
    # matmuls are declared float32r end-to-end so the BIR verifier sees
    # every producer rounding to fp32r.
    y0c_d = nc.dram_tensor("y0c", [D + 1, BC], FMM, kind="ExternalInput")
    w1t_d = nc.dram_tensor("w1t", [D, W], FMM, kind="ExternalInput")
    w2t_d = nc.dram_tensor("w2t", [W, W], FMM, kind="ExternalInput")
    w3t_d = nc.dram_tensor("w3t", [W, W], FMM, kind="ExternalInput")
    mts_d = nc.dram_tensor("mts", [W, NS * W], FMM, kind="ExternalInput")
    gam_d = nc.dram_tensor("gam", [W, NS], F32, kind="ExternalInput")
    b2_d = nc.dram_tensor("b2c", [W, 1], F32, kind="ExternalInput")
    b3_d = nc.dram_tensor("b3c", [W, 1], F32, kind="ExternalInput")
    amat_d = nc.dram_tensor("amat", [D + 1, 2 * T], FMM, kind="ExternalInput")
    bmat_d = nc.dram_tensor("bmat", [W, NS * 2 * T], FMM, kind="ExternalInput")
    yout_d = nc.dram_tensor("yout", [2 * T, BC], F32, kind="ExternalOutput")

    with tile.TileContext(nc) as tc:
        with (
            tc.tile_pool(name="const", bufs=1) as cpool,
            tc.tile_pool(name="h1", bufs=3) as h1pool,
            tc.tile_pool(name="h2", bufs=3) as h2pool,
            tc.tile_pool(name="h3", bufs=3) as h3pool,
            tc.tile_pool(name="yo", bufs=2) as ypool,
            tc.tile_pool(name="pp", bufs=1, space="PSUM") as ppsum,
            tc.tile_pool(name="py", bufs=1, space="PSUM") as ypsum_pool,
            tc.tile_pool(name="pa0l", bufs=1, space="PSUM") as papool0l,
            tc.tile_pool(name="pa0h", bufs=1, space="PSUM") as papool0h,
            tc.tile_pool(name="pa1l", bufs=1, space="PSUM") as papool1l,
            tc.tile_pool(name="pa1h", bufs=1, space="PSUM") as papool1h,
        ):
            papool = [[papool0l, papool0h], [papool1l, papool1h]]
            # Prefetch the ACT function-table load: walrus inserts it
            # before the first ACTIVATE in program order, so issue a tiny
            # dummy lrelu with no data dependencies at t~0.
            warm = cpool.tile([1, 1], F32)
            nc.vector.memset(warm[:], 0.0)
            warm2 = cpool.tile([1, 1], F32)
            nc.scalar.activation(warm2[:], warm[:], AF.Lrelu, bias=0.0,
                                 scale=1.0, alpha=0.01)

            # ---- constants into SBUF ----
            y0t = cpool.tile([D + 1, BC], FMM)
            w1t = cpool.tile([D, W], FMM)
            w2t = cpool.tile([W, W], FMM)
            w3t = cpool.tile([W, W], FMM)
            mts = cpool.tile([W, NS * W], FMM)
            gam = cpool.tile([W, NS], F32)
            b2c = cpool.tile([W, 1], F32)
            b3c = cpool.tile([W, 1], F32)
            amat = cpool.tile([D + 1, 2 * T], FMM)
            bmat = cpool.tile([W, NS * 2 * T], FMM)
            for i, (t_sb, t_dr) in enumerate((
                (y0t, y0c_d), (w1t, w1t_d), (w2t, w2t_d), (w3t, w3t_d),
                (gam, gam_d), (b2c, b2_d),
                (b3c, b3_d), (amat, amat_d),
            )):
                (nc.sync if i % 2 == 0 else nc.gpsimd).dma_start(t_sb[:], t_dr[:])
            # the two big tables: quarter them and spread across both queues
            for big_sb, big_dr, ncols in ((mts, mts_d, NS * W), (bmat, bmat_d, NS * 2 * T)):
                q = ncols // 4
                for i in range(4):
                    sl = slice(i * q, (i + 1) * q) if i < 3 else slice(3 * q, ncols)
                    (nc.sync if i % 2 == 0 else nc.gpsimd).dma_start(
                        big_sb[:, sl], big_dr[:, sl]
                    )

            # ---- persistent PSUM state: P (the rescaled a1 accumulator) ----
            P = [
                ppsum.tile([W, CH], F32, name=f"P{c}", tag=f"P{c}")
                for c in range(NCH)
            ]
            Y = [
                ypsum_pool.tile([2 * T, CH], F32, name=f"Y{c}", tag=f"Y{c}")
                for c in range(NCH)
            ]
            for c in range(NCH):
                nc.tensor.matmul(
                    P[c][:], w1t[:], y0t[0:D, c * CH : (c + 1) * CH],
                    start=True, stop=True,
                )
                nc.tensor.matmul(
                    Y[c][:], amat[:], y0t[:, c * CH : (c + 1) * CH],
                    start=True, stop=True,
                )

            # ---- integration loop ----
            lrelu_op = _register_lrelu_op()

            def dve_lrelu(out_ap, in_ap, bias_ap, scale):
                nc.vector._custom_dve(
                    lrelu_op, out=out_ap, in0=in_ap,
                    s0=bias_ap, s1=float(scale), imm2=0.01,
                )

            HF = CH // 2  # 256: lo/hi column halves of each chunk

            for n in range(NS):
                en = e_scales[n]
                # emit layer-by-layer waves across both chunks so the
                # scheduler phase-locks the chunks around ACT/DVE
                h1s, h2s, h3s, a2s, a3s = [], [], [], [], []
                for c in range(NCH):
                    h1 = h1pool.tile([W, CH], FMM, name=f"h1_{n}_{c}", tag="h1")
                    if c == 0:
                        nc.scalar.activation(
                            h1[:], P[c][:], AF.Lrelu,
                            bias=gam[:, n : n + 1], scale=float(en), alpha=0.01,
                        )
                    else:
                        dve_lrelu(h1[:], P[c][:], gam[:, n : n + 1], en)
                    h1s.append(h1)
                for c in range(NCH):
                    a2s.append([])
                    for half in range(2):
                        hs = slice(half * HF, (half + 1) * HF)
                        a2 = papool[c][half].tile(
                            [W, HF], F32, name=f"a2_{n}_{c}_{half}", tag="a"
                        )
                        nc.tensor.matmul(
                            a2[:], w2t[:], h1s[c][:, hs], start=True, stop=True
                        )
                        a2s[c].append(a2)
                for c in range(NCH):
                    h2 = h2pool.tile([W, CH], FMM, name=f"h2_{n}_{c}", tag="h2")
                    for half in range(2):
                        hs = slice(half * HF, (half + 1) * HF)
                        if half == 0:
                            nc.scalar.activation(
                                h2[:, hs], a2s[c][half][:], AF.Lrelu,
                                bias=b2c[:], scale=1.0, alpha=0.01,
                            )
                        else:
                            dve_lrelu(h2[:, hs], a2s[c][half][:], b2c[:], 1.0)
                    h2s.append(h2)
                for c in range(NCH):
                    a3s.append([])
                    for half in range(2):
                        hs = slice(half * HF, (half + 1) * HF)
                        a3 = papool[c][half].tile(
                            [W, HF], F32, name=f"a3_{n}_{c}_{half}", tag="a"
                        )
                        nc.tensor.matmul(
                            a3[:], w3t[:], h2s[c][:, hs], start=True, stop=True
                        )
                        a3s[c].append(a3)
                for c in range(NCH):
                    h3 = h3pool.tile([W, CH], FMM, name=f"h3_{n}_{c}", tag="h3")
                    for half in range(2):
                        hs = slice(half * HF, (half + 1) * HF)
                        if half == 0:
                            nc.scalar.activation(
                                h3[:, hs], a3s[c][half][:], AF.Lrelu,
                                bias=b3c[:], scale=1.0, alpha=0.01,
                            )
                        else:
                            dve_lrelu(h3[:, hs], a3s[c][half][:], b3c[:], 1.0)
                    h3s.append(h3)
                for c in range(NCH):
                    if n < NS - 1:
                        nc.tensor.matmul(
                            P[c][:], mts[:, n * W : (n + 1) * W], h3s[c][:],
                            start=False, stop=True,
                        )
                for c in range(NCH):
                    nc.tensor.matmul(
                        Y[c][:], bmat[:, n * 2 * T : (n + 1) * 2 * T], h3s[c][:],
                        start=False, stop=True,
                    )

            # ---- clip and store the trajectory ----
            for c in range(NCH):
                cs = slice(c * CH, (c + 1) * CH)
                yo = ypool.tile([2 * T, CH], F32, tag="yo")
                nc.vector.tensor_scalar(
                    yo[:], Y[c][:], -CAP, CAP, ALU.max, ALU.min
                )
                nc.sync.dma_start(yout_d[:, cs], yo[:])

    nc.compile()
    return nc


_NC_CACHE = None


def _get_module(e_scales):
    global _NC_CACHE
    if _NC_CACHE is None:
        _NC_CACHE = _build_module(e_scales)
    return _NC_CACHE


def kernel(ts, y0, W1, b1, W2, b2, W3, b3, W4, b4):
    ts = np.asarray(ts, np.float32)
    y0 = np.asarray(y0, np.float32)
    W1 = np.asarray(W1, np.float32)
    b1 = np.asarray(b1, np.float32)
    W2 = np.asarray(W2, np.float32)
    b2 = np.asarray(b2, np.float32)
    W3 = np.asarray(W3, np.float32)
    b3 = np.asarray(b3, np.float32)
    W4 = np.asarray(W4, np.float32)
    b4 = np.asarray(b4, np.float32)

    gam, mts, Amat, Bmat, e_scales = _host_tables(ts, W1, b1, W4, b4)
    nc = _get_module(e_scales)

    y0t_all = np.ascontiguousarray(y0.T)  # [2, 8192]
    shared = {
        "w1t": np.ascontiguousarray(W1.T),
        "w2t": np.ascontiguousarray(W2.T),
        "w3t": np.ascontiguousarray(W3.T),
        "mts": mts,
        "gam": gam,
        "b2c": b2.reshape(W, 1).copy(),
        "b3c": b3.reshape(W, 1).copy(),
        "amat": Amat,
        "bmat": Bmat,
    }
    in_maps = []
    for i in range(NCORES):
        m = dict(shared)
        m["y0c"] = np.ascontiguousarray(
            np.vstack(
                [y0t_all[:, i * BC : (i + 1) * BC], np.ones((1, BC), np.float32)]
            )
        )
        in_maps.append(m)

    from concourse.bass_utils import run_bass_kernel_spmd

    res = run_bass_kernel_spmd(nc, in_maps, core_ids=list(range(NCORES)))

    ys = np.empty((T, B, D), np.float32)
    for i in range(NCORES):
        arr = res.results[i]["yout"]  # [128, 1024]
        ys[:, i * BC : (i + 1) * BC, :] = arr.reshape(T, D, BC).transpose(0, 2, 1)
    return ys
